# revision 30
# baseline (speedup 1.0000x reference)
"""Causal multi-head attention on 8 trn2 NeuronCores.

Sharding: core c -> (batch b = c//2, head-group hg = c%2).
Each head-group owns 8 of the 16 heads (512 of the 1024 embed dims after
the head split). Per core:
  - qT, kT   = (x[b] @ Wq_hg)^T (wq pre-scaled by 1/sqrt(d)), (x[b] @ Wk_hg)^T
  - v        = x[b] @ Wv_hg, packed per-head with a ones column -> va
  - scoresT  = kT.T-contract [krows, qrows] tiles; exp (-> bf16); causal mask
  - av       = va^T @ ex -> [128, qcols]: rows 0:64 context, rows 64:128 the
               softmax denominator replicated 64x (va carries 64 ones cols, so
               the broadcast falls out of the matmul for free)
  - ctxT     = av[0:64] * approx_recip(av[64:128])
  - partial  = ctxT.T @ Wproj_hg (+ bproj on hg==0 cores only)
Host: out[b] = partial(b,0) + partial(b,1).

v2 schedule (vs baseline, 140.7us -> ~119us): the whole kernel is one
software-pipelined stream keyed to two facts from the baseline trace:
(1) the PE streams at its pure warm rate whenever its queue is
non-empty, and (2) scalar exp (~50us) is the only consumer that can
starve it. So:
  - v(rt0-2) + qk(ct0) interleave per-et so compute tracks the DMA
    stream and the first exp starts ~8us earlier; one PSUM buf is kept
    free so v2(3) bridges the post-loop copy latency.
  - input DMA: xt in halves, only ct0 columns of wq/wk early (3.5MB
    early stream), the remaining 4MB strictly behind it in-queue so it
    cannot steal bandwidth from the et-loop's feed.
  - score MMs for the two heads of a ct-tile issue interleaved at row
    groups (0,0)/(64,0) -> they stream concurrently (K=64 packing).
  - emission interleaves {score units, av chains, qk/v2/proj chunks} so
    there is >= the exp drain time of PE work between a score unit and
    anything that waits on its exp; PE never idles -> HAM stays warm
    for the whole kernel (one cold ramp instead of four).
  - causal masks moved scalar/vector -> gpsimd (otherwise idle).
  - all PSUM->SBUF copies moved off scalar (exp-only) onto vector.
"""

import sys

try:
    import concourse.bass as bass  # noqa: F401
except Exception:
    sys.path.insert(0, "/opt/trn_rl_repo")

import numpy as np
import ml_dtypes

import concourse.bass as bass
import concourse.mybir as mybir
import concourse.tile as tile
from concourse import bacc
from concourse.bass_utils import run_bass_kernel_spmd

F32 = mybir.dt.float32
F32R = mybir.dt.float32r
BF16 = mybir.dt.bfloat16
AF = mybir.ActivationFunctionType
BF_NP = ml_dtypes.bfloat16

B, S, E = 4, 1024, 1024
H, D = 16, 64
HG = 2              # head groups (cores per batch)
HPG = H // HG       # 8 heads per group
EG = HPG * D        # 512 embed cols per group
P = 128
ET = E // P         # 8 embed tiles
RT = S // P         # 8 row tiles
CT = EG // P        # 4 col tiles of the group's q/k
QCH = 512           # q-chunk (moving free dim)
NQC = S // QCH      # 2 q chunks
SCALE = 1.0 / np.sqrt(D)


def _emit(nc, tc, with_bias):
    xt = nc.dram_tensor("xt", [ET, P, S], BF16, kind="ExternalInput")
    wq = nc.dram_tensor("wq", [ET, P, EG], BF16, kind="ExternalInput")
    wk = nc.dram_tensor("wk", [ET, P, EG], BF16, kind="ExternalInput")
    wv = nc.dram_tensor("wv", [ET, P, EG], BF16, kind="ExternalInput")
    wp = nc.dram_tensor("wp", [CT, P, E], BF16, kind="ExternalInput")
    bq = nc.dram_tensor("bq", [P, CT], F32, kind="ExternalInput")
    bk = nc.dram_tensor("bk", [P, CT], F32, kind="ExternalInput")
    bv = nc.dram_tensor("bv", [1, EG], BF16, kind="ExternalInput")
    bp = nc.dram_tensor("bp", [1, E], BF16, kind="ExternalInput")
    mask = nc.dram_tensor("mask", [P, P], BF16, kind="ExternalInput")
    ones_in = nc.dram_tensor("ones", [1, P], BF16, kind="ExternalInput")
    vones_in = nc.dram_tensor("vones", [P, HPG * D], BF16, kind="ExternalInput")
    out = nc.dram_tensor("out", [S, E], BF16, kind="ExternalOutput")

    with (
        tc.tile_pool(name="xt", bufs=1) as p_xt,
        tc.tile_pool(name="wqkv", bufs=1) as p_w,
        tc.tile_pool(name="wp", bufs=1) as p_wp,
        tc.tile_pool(name="qt", bufs=1) as p_qt,
        tc.tile_pool(name="kt", bufs=1) as p_kt,
        tc.tile_pool(name="vaug", bufs=1) as p_va,
        tc.tile_pool(name="ctxT", bufs=1) as p_ctx,
        tc.tile_pool(name="exps", bufs=24) as p_exp,
        tc.tile_pool(name="small", bufs=1) as p_sm,
        tc.tile_pool(name="recip", bufs=4) as p_rc,
        tc.tile_pool(name="osb", bufs=4) as p_osb,
        tc.tile_pool(name="pr", bufs=2, space="PSUM") as p_pr,   # 2x[128,1024]f32
        tc.tile_pool(name="ps1", bufs=4, space="PSUM") as p_ps,  # 4x[128,512]f32
    ):
        # ---- SBUF input tiles ----
        xt_t = [p_xt.tile([P, S], BF16, tag=f"xt{et}", name=f"xt{et}")
                for et in range(ET)]
        wq_t = [p_w.tile([P, EG], BF16, tag=f"wq{et}", name=f"wq{et}")
                for et in range(ET)]
        wk_t = [p_w.tile([P, EG], BF16, tag=f"wk{et}", name=f"wk{et}")
                for et in range(ET)]
        wv_t = [p_w.tile([P, EG], BF16, tag=f"wv{et}", name=f"wv{et}")
                for et in range(ET)]
        wp_t = [p_wp.tile([P, E], BF16, tag=f"wp{et}", name=f"wp{et}")
                for et in range(CT)]

        # ---- HAM warm-up: ~8 garbage MMs (on a vector-memset tile, so no
        # DMA dependency) fill the idle window between the preamble and the
        # first real matmul. ~3.4us of sustained PE activity flips the clock
        # gate to 8/8 before the real stream begins instead of ~8us into it.
        junk = p_sm.tile([P, QCH], BF16, tag="junk", name="junk")
        nc.vector.memset(junk[:], 0)
        wmps = p_pr.tile([P, S], F32, tag="pr", name="warmps")
        for i in range(8):
            nc.tensor.matmul(
                wmps[:, (i % 2) * QCH:(i % 2 + 1) * QCH],
                junk[:, 0:P], junk[:],
                start=True, stop=True)

        # ---- input DMA: per-et groups, xt in halves, so the v1/qk0 et-loop
        # tracks the stream; spread across the three DMA-capable queues.
        # Only ct=0 columns of wq/wk ship early (all qk0 needs): the early
        # stream is 3.5MB instead of 6MB, so compute tracks DMA ~5us sooner.
        # Queue plan (transfers within a queue are ordered, queues share HBM
        # bandwidth ~equally): sync: xt halves 0 (1MB); scalar: wv (1MB);
        # gpsimd: wq/wk ct0 + xt halves 1 (1.5MB). The late 4MB (wq/wk ct1-3,
        # wp) sits strictly BEHIND the early stream in its queue so it cannot
        # steal bandwidth from the et-loop's feed.
        nc.sync.dma_start(xt_t[0][:, 0:2 * P], xt[0][:, 0:2 * P])
        nc.sync.dma_start(xt_t[0][:, 2 * P:QCH], xt[0][:, 2 * P:QCH])
        for et in range(ET):
            nc.scalar.dma_start(wv_t[et][:], wv[et])
            if et > 0:
                nc.sync.dma_start(xt_t[et][:, 0:QCH], xt[et][:, 0:QCH])
            nc.gpsimd.dma_start(wk_t[et][:, 0:P], wk[et][:, 0:P])
            nc.gpsimd.dma_start(wq_t[et][:, 0:P], wq[et][:, 0:P])
            nc.gpsimd.dma_start(xt_t[et][:, QCH:S], xt[et][:, QCH:S])
        mask_sb = p_sm.tile([P, P], BF16, tag="mask", name="maskt")
        nc.gpsimd.dma_start(mask_sb[:], mask[:])
        vones_sb = p_sm.tile([P, HPG * D], BF16, tag="vones", name="vones")
        nc.gpsimd.dma_start(vones_sb[:], vones_in[:])
        # late stream: remaining wq/wk columns (ct 1..3) + wp
        for et in range(ET):
            nc.scalar.dma_start(wq_t[et][:, P:EG], wq[et][:, P:EG])
            nc.gpsimd.dma_start(wk_t[et][:, P:EG], wk[et][:, P:EG])
        for et in range(CT):
            nc.sync.dma_start(wp_t[et][:], wp[et])
        if with_bias:
            ones_sb = p_sm.tile([1, P], BF16, tag="ones", name="ones")
            nc.sync.dma_start(ones_sb[:], ones_in[:])
            bq_sb = p_sm.tile([P, CT], F32, tag="bq", name="bqt")
            nc.sync.dma_start(bq_sb[:], bq[:])
            bk_sb = p_sm.tile([P, CT], F32, tag="bk", name="bkt")
            nc.sync.dma_start(bk_sb[:], bk[:])
            bv_sb = p_sm.tile([1, EG], BF16, tag="bv", name="bvt")
            nc.sync.dma_start(bv_sb[:], bv[:])
            bp_sb = p_sm.tile([1, E], BF16, tag="bp", name="bpt")
            nc.sync.dma_start(bp_sb[:], bp[:])
        else:
            ones_sb = bq_sb = bk_sb = bv_sb = bp_sb = None

        # ---- long-lived SBUF result tiles ----
        qT_t = [p_qt.tile([P, S], BF16, tag=f"qt{ct}", name=f"qt{ct}")
                for ct in range(CT)]
        kT_t = [p_kt.tile([P, S], BF16, tag=f"kt{ct}", name=f"kt{ct}")
                for ct in range(CT)]
        va_t = [p_va.tile([P, HPG * 2 * D], BF16, tag=f"va{rt}", name=f"va{rt}")
                for rt in range(RT)]
        ctx_t = [p_ctx.tile([P, S], BF16, tag=f"cx{i}", name=f"cx{i}")
                 for i in range(CT)]

        # ones-halves of va (vector, early while otherwise idle)
        for rt in range(RT):
            va3 = va_t[rt][:].rearrange("p (h d) -> p h d", h=HPG)
            nc.vector.tensor_copy(
                va3[:, :, D:2 * D],
                vones_sb[:].rearrange("p (h d) -> p h d", h=HPG))

        def v_fin(rt, vps):
            if with_bias:
                nc.tensor.matmul(
                    vps, ones_sb[0:1, 0:P], bv_sb[0:1, :],
                    start=False, stop=True)
            nc.vector.tensor_copy(
                va_t[rt][:].rearrange("p (h d) -> p h d", h=HPG)[:, :, 0:D],
                vps.rearrange("p (h d) -> p h d", h=HPG))

        # ================= phase V1 + QK0: interleaved per-et =============
        # v1 covers rt 0..2 only: 3 + 2 + 2 PSUM tiles leaves one p_ps buf
        # free so v2(3) can start the moment the et-loop drains, covering
        # the vector copies that everything else gates on.
        NV1 = 3
        v_ps = [p_ps.tile([P, EG], F32, tag="ps", name=f"vps{rt}")
                for rt in range(NV1)]
        q0 = p_pr.tile([P, S], F32, tag="pr", name="q0pr")
        k0 = p_pr.tile([P, S], F32, tag="pr", name="k0pr")
        for et in range(ET):
            for rt in range(NV1):
                nc.tensor.matmul(
                    v_ps[rt][:], xt_t[et][:, rt * P:(rt + 1) * P], wv_t[et][:],
                    start=(et == 0), stop=(et == ET - 1 and not with_bias))
            for rc in range(NQC):  # rc-outer: xt half 1 consumed last
                nc.tensor.matmul(
                    q0[:, rc * QCH:(rc + 1) * QCH],
                    wq_t[et][:, 0:P], xt_t[et][:, rc * QCH:(rc + 1) * QCH],
                    start=(et == 0), stop=(et == ET - 1))
                nc.tensor.matmul(
                    k0[:, rc * QCH:(rc + 1) * QCH],
                    wk_t[et][:, 0:P], xt_t[et][:, rc * QCH:(rc + 1) * QCH],
                    start=(et == 0), stop=(et == ET - 1))

        def qk_copy(dst, src, bias_sb, ct):
            if with_bias:
                nc.vector.tensor_scalar_add(dst, src, bias_sb[:, ct:ct + 1])
            else:
                nc.vector.tensor_copy(dst, src)

        # scores gate on these two copies -> they go first on vector
        qk_copy(qT_t[0][:, 0:S], q0[:], bq_sb, 0)
        qk_copy(kT_t[0][:, 0:S], k0[:], bk_sb, 0)
        for rt in range(NV1):
            v_fin(rt, v_ps[rt][:])

        # ================= chunked units for the pipelined stream =========
        def v2_chunk(rt):
            vps = p_ps.tile([P, EG], F32, tag="ps", name=f"vps{rt}")
            for et in range(ET):
                nc.tensor.matmul(
                    vps[:], xt_t[et][:, rt * P:(rt + 1) * P], wv_t[et][:],
                    start=(et == 0), stop=(et == ET - 1 and not with_bias))
            v_fin(rt, vps[:])

        qk_state = {}

        def qk_part(ct, which, half):
            # which: 'q'|'k'; half 0 -> et 0..3, 1 -> et 4..7 (+copies)
            wt = wq_t if which == 'q' else wk_t
            if half == 0:
                qk_state[(ct, which)] = [
                    p_ps.tile([P, QCH], F32, tag="ps", name=f"{which}{ct}ps{rc}")
                    for rc in range(NQC)]
            accs = qk_state[(ct, which)]
            for et in range(half * 4, half * 4 + 4):
                for rc in range(NQC):
                    nc.tensor.matmul(
                        accs[rc][:], wt[et][:, ct * P:(ct + 1) * P],
                        xt_t[et][:, rc * QCH:(rc + 1) * QCH],
                        start=(et == 0), stop=(et == ET - 1))
            if half == 1:
                dstt = qT_t if which == 'q' else kT_t
                bias_sb = bq_sb if which == 'q' else bk_sb
                for rc in range(NQC):
                    qk_copy(dstt[ct][:, rc * QCH:(rc + 1) * QCH],
                            accs[rc][:], bias_sb, ct)

        # score units: ex_store[(qc, p, kt, h)] = (ex AP slice, off)
        ex_store = {}

        def s_unit(qc, p, kp):
            """Scores for head pair p, k-tiles (2kp, 2kp+1), q-window qc.
            The two heads' MMs interleave at row groups 0/64 and stream
            concurrently; per-head packed PSUM [0:na]+[na:na+nb]; one exp
            activate per head; diagonal masks on gpsimd."""
            kts = (2 * kp, 2 * kp + 1)
            offs = [max(0, kt - qc * (QCH // P)) * P for kt in kts]
            ns = [QCH - o for o in offs]
            cols = [0, ns[0]]
            w = ns[0] + ns[1]
            prs = [p_pr.tile([P, S], F32, tag="pr", name=f"pr{qc}{p}{kp}{h}")
                   for h in range(2)]
            exs = [p_exp.tile([P, S], BF16, tag="ex", name=f"ex{qc}{p}{kp}{h}")
                   for h in range(2)]
            for i, kt in enumerate(kts):
                for h in range(2):
                    hb = h * D
                    nc.tensor.matmul(
                        prs[h][:, cols[i]:cols[i] + ns[i]],
                        kT_t[p][hb:hb + D, kt * P:(kt + 1) * P],
                        qT_t[p][hb:hb + D,
                                qc * QCH + offs[i]:(qc + 1) * QCH],
                        start=True, stop=True,
                        tile_position=(hb, 0))
            for h in range(2):
                nc.scalar.activation(exs[h][:, 0:w], prs[h][:, 0:w], AF.Exp)
            for h in range(2):  # h-outer: h0 masks drain during h1's exp
                for i, kt in enumerate(kts):
                    if kt >= qc * (QCH // P):  # diagonal-crossing tile
                        nc.gpsimd.tensor_mul(
                            exs[h][:, cols[i]:cols[i] + P],
                            exs[h][:, cols[i]:cols[i] + P], mask_sb[:])
            for i, kt in enumerate(kts):
                for h in range(2):
                    ex_store[(qc, p, kt, h)] = (
                        exs[h][:, cols[i]:cols[i] + ns[i]], offs[i])

        def av_unit(qc, p, h):
            """Context accumulation for one head (global head 2p+h)."""
            gh = 2 * p + h
            hb = h * D
            av = p_ps.tile([P, QCH], F32, tag="ps", name=f"av{qc}{p}{h}")
            n_kt = (qc + 1) * (QCH // P)
            for kt in range(n_kt):
                ex_ap, off = ex_store[(qc, p, kt, h)]
                nc.tensor.matmul(
                    av[:, off:QCH],
                    va_t[kt][:, gh * 2 * D:(gh + 1) * 2 * D],
                    ex_ap,
                    start=(kt == 0), stop=(kt == n_kt - 1))
            den_sb = p_rc.tile([D, QCH], F32, tag="den", name="den")
            nc.vector.tensor_copy(den_sb[:], av[D:2 * D, :])
            rcb = p_rc.tile([D, QCH], F32, tag="rc", name="rc")
            nc.vector.reciprocal_approx_fast(rcb[:], den_sb[:])
            nc.vector.tensor_mul(
                ctx_t[p][hb:hb + D, qc * QCH:(qc + 1) * QCH],
                av[0:D, :], rcb[:])

        def project(rt):
            for cc in range(E // QCH):
                ps = p_ps.tile([P, QCH], F32, tag="ps", name=f"pj{rt}{cc}")
                for et in range(CT):
                    nc.tensor.matmul(
                        ps[:],
                        ctx_t[et][:, rt * P:(rt + 1) * P],
                        wp_t[et][:, cc * QCH:(cc + 1) * QCH],
                        start=(et == 0),
                        stop=(et == CT - 1 and not with_bias))
                if with_bias:
                    nc.tensor.matmul(
                        ps[:], ones_sb[0:1, 0:P],
                        bp_sb[0:1, cc * QCH:(cc + 1) * QCH],
                        start=False, stop=True)
                osb = p_osb.tile([P, QCH], BF16, tag="osb", name="osb")
                if (rt + cc) % 2 == 0:
                    nc.vector.tensor_copy(osb[:], ps[:])
                else:
                    nc.scalar.copy(osb[:], ps[:])
                # gpsimd is busy with masks mid-kernel but free in the tail
                eng = nc.gpsimd if rt >= 6 and (rt + cc) % 2 == 1 else nc.sync
                eng.dma_start(
                    out[rt * P:(rt + 1) * P, cc * QCH:(cc + 1) * QCH],
                    osb[:])

        # ================= the pipelined stream ===========================
        # block 0: att0/att1 scores of pair 0; fillers v2 + qk1.
        # v2(3) leads into the free PSUM buf while vector runs the copies.
        v2_chunk(3)
        s_unit(0, 0, 0)
        v2_chunk(4)
        s_unit(0, 0, 1)
        v2_chunk(5)
        s_unit(1, 0, 0)
        v2_chunk(6)
        s_unit(1, 0, 1)
        v2_chunk(7)
        qk_part(1, 'k', 0)
        s_unit(1, 0, 2)
        qk_part(1, 'k', 1)
        s_unit(1, 0, 3)
        qk_part(1, 'q', 0)
        qk_part(1, 'q', 1)
        av_unit(0, 0, 0)
        av_unit(0, 0, 1)

        # block 1: pair 1 scores; fillers qk2 + av of pair 0
        s_unit(0, 1, 0)
        qk_part(2, 'k', 0)
        s_unit(0, 1, 1)
        av_unit(1, 0, 0)
        s_unit(1, 1, 0)
        av_unit(1, 0, 1)
        s_unit(1, 1, 1)
        qk_part(2, 'k', 1)
        s_unit(1, 1, 2)
        qk_part(2, 'q', 0)
        s_unit(1, 1, 3)
        qk_part(2, 'q', 1)
        av_unit(0, 1, 0)
        av_unit(0, 1, 1)

        # block 2: pair 2 scores; fillers qk3 + av(1,1,h0) (h1 deferred
        # to block 3, which otherwise runs out of exp-independent work)
        s_unit(0, 2, 0)
        qk_part(3, 'k', 0)
        s_unit(0, 2, 1)
        av_unit(1, 1, 0)
        s_unit(1, 2, 0)
        qk_part(3, 'k', 1)
        s_unit(1, 2, 1)
        qk_part(3, 'q', 0)
        s_unit(1, 2, 2)
        qk_part(3, 'q', 1)
        s_unit(1, 2, 3)
        av_unit(0, 2, 0)
        s_unit(1, 3, 0)   # pulled forward: exps drain in block2's slack
        av_unit(0, 2, 1)
        s_unit(1, 3, 1)

        # block 3: last pair; fillers av(1,1,h1) + av(1,2) + proj(0..3)
        s_unit(0, 3, 0)
        av_unit(1, 1, 1)
        s_unit(0, 3, 1)
        av_unit(1, 2, 0)
        av_unit(1, 2, 1)
        av_unit(0, 3, 0)
        av_unit(0, 3, 1)
        project(0)
        s_unit(1, 3, 2)
        project(1)
        s_unit(1, 3, 3)
        project(2)
        project(3)

        # tail
        av_unit(1, 3, 0)
        av_unit(1, 3, 1)
        for rt in range(4, RT):
            project(rt)


def build_nc(with_bias=False):
    nc = bacc.Bacc("TRN2", target_bir_lowering=False, debug=False)
    with tile.TileContext(nc) as tc, nc.allow_low_precision(
        reason="bf16 matmul operands with fp32 accumulate; approx reciprocal"
    ):
        _emit(nc, tc, with_bias)
    nc.compile()
    return nc


def make_in_maps(x, Wqkv, bqkv, Wproj, bproj):
    x = np.asarray(x, dtype=np.float32)
    Wqkv = np.asarray(Wqkv, dtype=np.float32)
    bqkv = np.asarray(bqkv, dtype=np.float32)
    Wproj = np.asarray(Wproj, dtype=np.float32)
    bproj = np.asarray(bproj, dtype=np.float32)
    mask = np.triu(np.ones((P, P), dtype=np.float32))  # [k, q]: k <= q
    in_maps = []
    for c in range(8):
        b, hg = c // 2, c % 2
        g = slice(hg * EG, (hg + 1) * EG)
        in_maps.append({
            "xt": np.ascontiguousarray(x[b].T).reshape(ET, P, S).astype(BF_NP),
            "wq": np.ascontiguousarray(
                Wqkv[:, 0 * E:1 * E][:, g] * SCALE).reshape(ET, P, EG).astype(BF_NP),
            "wk": np.ascontiguousarray(
                Wqkv[:, 1 * E:2 * E][:, g]).reshape(ET, P, EG).astype(BF_NP),
            "wv": np.ascontiguousarray(
                Wqkv[:, 2 * E:3 * E][:, g]).reshape(ET, P, EG).astype(BF_NP),
            "wp": np.ascontiguousarray(Wproj[g, :]).reshape(CT, P, E).astype(BF_NP),
            "bq": np.ascontiguousarray(
                (bqkv[0 * E:1 * E][g] * SCALE).reshape(CT, P).T),
            "bk": np.ascontiguousarray(
                bqkv[1 * E:2 * E][g].reshape(CT, P).T),
            "bv": bqkv[2 * E:3 * E][g].reshape(1, EG).astype(BF_NP),
            "bp": (bproj if hg == 0 else np.zeros_like(bproj)
                   ).reshape(1, E).astype(BF_NP),
            "mask": mask.astype(BF_NP),
            "ones": np.ones((1, P), dtype=BF_NP),
            "vones": np.ones((P, HPG * D), dtype=BF_NP),
        })
    return in_maps


def kernel(x, Wqkv, bqkv, Wproj, bproj):
    with_bias = bool(
        np.any(np.asarray(bqkv)) or np.any(np.asarray(bproj)))
    nc = build_nc(with_bias)
    in_maps = make_in_maps(x, Wqkv, bqkv, Wproj, bproj)
    res = run_bass_kernel_spmd(nc, in_maps, list(range(8))).results
    out = np.zeros((B, S, E), dtype=np.float32)
    for c in range(8):
        out[c // 2] += res[c]["out"]
    return out


# revision 31
# speedup vs baseline: 1.0034x; 1.0034x over previous
"""Causal multi-head attention on 8 trn2 NeuronCores.

Sharding: core c -> (batch b = c//2, head-group hg = c%2).
Each head-group owns 8 of the 16 heads (512 of the 1024 embed dims after
the head split). Per core:
  - qT, kT   = (x[b] @ Wq_hg)^T (wq pre-scaled by 1/sqrt(d)), (x[b] @ Wk_hg)^T
  - v        = x[b] @ Wv_hg, packed per-head with a ones column -> va
  - scoresT  = kT.T-contract [krows, qrows] tiles; exp (-> bf16); causal mask
  - av       = va^T @ ex -> [128, qcols]: rows 0:64 context, rows 64:128 the
               softmax denominator replicated 64x (va carries 64 ones cols, so
               the broadcast falls out of the matmul for free)
  - ctxT     = av[0:64] * approx_recip(av[64:128])
  - partial  = ctxT.T @ Wproj_hg (+ bproj on hg==0 cores only)
Host: out[b] = partial(b,0) + partial(b,1).

v2 schedule (vs baseline, 140.7us -> ~119us): the whole kernel is one
software-pipelined stream keyed to two facts from the baseline trace:
(1) the PE streams at its pure warm rate whenever its queue is
non-empty, and (2) scalar exp (~50us) is the only consumer that can
starve it. So:
  - v(rt0-2) + qk(ct0) interleave per-et so compute tracks the DMA
    stream and the first exp starts ~8us earlier; one PSUM buf is kept
    free so v2(3) bridges the post-loop copy latency.
  - input DMA: xt in halves, only ct0 columns of wq/wk early (3.5MB
    early stream), the remaining 4MB strictly behind it in-queue so it
    cannot steal bandwidth from the et-loop's feed.
  - score MMs for the two heads of a ct-tile issue interleaved at row
    groups (0,0)/(64,0) -> they stream concurrently (K=64 packing).
  - emission interleaves {score units, av chains, qk/v2/proj chunks} so
    there is >= the exp drain time of PE work between a score unit and
    anything that waits on its exp; PE never idles -> HAM stays warm
    for the whole kernel (one cold ramp instead of four).
  - causal masks moved scalar/vector -> gpsimd (otherwise idle).
  - all PSUM->SBUF copies moved off scalar (exp-only) onto vector.
"""

import sys

try:
    import concourse.bass as bass  # noqa: F401
except Exception:
    sys.path.insert(0, "/opt/trn_rl_repo")

import numpy as np
import ml_dtypes

import concourse.bass as bass
import concourse.mybir as mybir
import concourse.tile as tile
from concourse import bacc
from concourse.bass_utils import run_bass_kernel_spmd

F32 = mybir.dt.float32
F32R = mybir.dt.float32r
BF16 = mybir.dt.bfloat16
AF = mybir.ActivationFunctionType
BF_NP = ml_dtypes.bfloat16

B, S, E = 4, 1024, 1024
H, D = 16, 64
HG = 2              # head groups (cores per batch)
HPG = H // HG       # 8 heads per group
EG = HPG * D        # 512 embed cols per group
P = 128
ET = E // P         # 8 embed tiles
RT = S // P         # 8 row tiles
CT = EG // P        # 4 col tiles of the group's q/k
QCH = 512           # q-chunk (moving free dim)
NQC = S // QCH      # 2 q chunks
SCALE = 1.0 / np.sqrt(D)


def _emit(nc, tc, with_bias):
    xt = nc.dram_tensor("xt", [ET, P, S], BF16, kind="ExternalInput")
    wq = nc.dram_tensor("wq", [ET, P, EG], BF16, kind="ExternalInput")
    wk = nc.dram_tensor("wk", [ET, P, EG], BF16, kind="ExternalInput")
    wv = nc.dram_tensor("wv", [ET, P, EG], BF16, kind="ExternalInput")
    wp = nc.dram_tensor("wp", [CT, P, E], BF16, kind="ExternalInput")
    bq = nc.dram_tensor("bq", [P, CT], F32, kind="ExternalInput")
    bk = nc.dram_tensor("bk", [P, CT], F32, kind="ExternalInput")
    bv = nc.dram_tensor("bv", [1, EG], BF16, kind="ExternalInput")
    bp = nc.dram_tensor("bp", [1, E], BF16, kind="ExternalInput")
    mask = nc.dram_tensor("mask", [P, P], BF16, kind="ExternalInput")
    ones_in = nc.dram_tensor("ones", [1, P], BF16, kind="ExternalInput")
    vones_in = nc.dram_tensor("vones", [P, HPG * D], BF16, kind="ExternalInput")
    out = nc.dram_tensor("out", [S, E], BF16, kind="ExternalOutput")

    with (
        tc.tile_pool(name="xt", bufs=1) as p_xt,
        tc.tile_pool(name="wqkv", bufs=1) as p_w,
        tc.tile_pool(name="wp", bufs=1) as p_wp,
        tc.tile_pool(name="qt", bufs=1) as p_qt,
        tc.tile_pool(name="kt", bufs=1) as p_kt,
        tc.tile_pool(name="vaug", bufs=1) as p_va,
        tc.tile_pool(name="ctxT", bufs=1) as p_ctx,
        tc.tile_pool(name="exps", bufs=24) as p_exp,
        tc.tile_pool(name="small", bufs=1) as p_sm,
        tc.tile_pool(name="recip", bufs=4) as p_rc,
        tc.tile_pool(name="osb", bufs=4) as p_osb,
        tc.tile_pool(name="pr", bufs=2, space="PSUM") as p_pr,   # 2x[128,1024]f32
        tc.tile_pool(name="ps1", bufs=4, space="PSUM") as p_ps,  # 4x[128,512]f32
    ):
        # ---- SBUF input tiles ----
        xt_t = [p_xt.tile([P, S], BF16, tag=f"xt{et}", name=f"xt{et}")
                for et in range(ET)]
        wq_t = [p_w.tile([P, EG], BF16, tag=f"wq{et}", name=f"wq{et}")
                for et in range(ET)]
        wk_t = [p_w.tile([P, EG], BF16, tag=f"wk{et}", name=f"wk{et}")
                for et in range(ET)]
        wv_t = [p_w.tile([P, EG], BF16, tag=f"wv{et}", name=f"wv{et}")
                for et in range(ET)]
        wp_t = [p_wp.tile([P, E], BF16, tag=f"wp{et}", name=f"wp{et}")
                for et in range(CT)]

        # ---- input DMA: per-et groups, xt in halves, so the v1/qk0 et-loop
        # tracks the stream; spread across the three DMA-capable queues.
        # Only ct=0 columns of wq/wk ship early (all qk0 needs): the early
        # stream is 3.5MB instead of 6MB, so compute tracks DMA ~5us sooner.
        # Queue plan (transfers within a queue are ordered, queues share HBM
        # bandwidth ~equally): sync: xt halves 0 (1MB); scalar: wv (1MB);
        # gpsimd: wq/wk ct0 + xt halves 1 (1.5MB). The late 4MB (wq/wk ct1-3,
        # wp) sits strictly BEHIND the early stream in its queue so it cannot
        # steal bandwidth from the et-loop's feed.
        nc.sync.dma_start(xt_t[0][:, 0:2 * P], xt[0][:, 0:2 * P])
        nc.sync.dma_start(xt_t[0][:, 2 * P:QCH], xt[0][:, 2 * P:QCH])
        for et in range(ET):
            nc.scalar.dma_start(wv_t[et][:], wv[et])
            if et > 0:
                nc.sync.dma_start(xt_t[et][:, 0:QCH], xt[et][:, 0:QCH])
            nc.gpsimd.dma_start(wk_t[et][:, 0:P], wk[et][:, 0:P])
            nc.gpsimd.dma_start(wq_t[et][:, 0:P], wq[et][:, 0:P])
            nc.gpsimd.dma_start(xt_t[et][:, QCH:S], xt[et][:, QCH:S])
        mask_sb = p_sm.tile([P, P], BF16, tag="mask", name="maskt")
        nc.gpsimd.dma_start(mask_sb[:], mask[:])
        vones_sb = p_sm.tile([P, HPG * D], BF16, tag="vones", name="vones")
        nc.gpsimd.dma_start(vones_sb[:], vones_in[:])
        # late stream: remaining wq/wk columns (ct 1..3) + wp
        for et in range(ET):
            nc.scalar.dma_start(wq_t[et][:, P:EG], wq[et][:, P:EG])
            nc.gpsimd.dma_start(wk_t[et][:, P:EG], wk[et][:, P:EG])
        for et in range(CT):
            nc.sync.dma_start(wp_t[et][:], wp[et])
        if with_bias:
            ones_sb = p_sm.tile([1, P], BF16, tag="ones", name="ones")
            nc.sync.dma_start(ones_sb[:], ones_in[:])
            bq_sb = p_sm.tile([P, CT], F32, tag="bq", name="bqt")
            nc.sync.dma_start(bq_sb[:], bq[:])
            bk_sb = p_sm.tile([P, CT], F32, tag="bk", name="bkt")
            nc.sync.dma_start(bk_sb[:], bk[:])
            bv_sb = p_sm.tile([1, EG], BF16, tag="bv", name="bvt")
            nc.sync.dma_start(bv_sb[:], bv[:])
            bp_sb = p_sm.tile([1, E], BF16, tag="bp", name="bpt")
            nc.sync.dma_start(bp_sb[:], bp[:])
        else:
            ones_sb = bq_sb = bk_sb = bv_sb = bp_sb = None

        # ---- long-lived SBUF result tiles ----
        qT_t = [p_qt.tile([P, S], BF16, tag=f"qt{ct}", name=f"qt{ct}")
                for ct in range(CT)]
        kT_t = [p_kt.tile([P, S], BF16, tag=f"kt{ct}", name=f"kt{ct}")
                for ct in range(CT)]
        va_t = [p_va.tile([P, HPG * 2 * D], BF16, tag=f"va{rt}", name=f"va{rt}")
                for rt in range(RT)]
        ctx_t = [p_ctx.tile([P, S], BF16, tag=f"cx{i}", name=f"cx{i}")
                 for i in range(CT)]

        # ones-halves of va (vector, early while otherwise idle)
        for rt in range(RT):
            va3 = va_t[rt][:].rearrange("p (h d) -> p h d", h=HPG)
            nc.vector.tensor_copy(
                va3[:, :, D:2 * D],
                vones_sb[:].rearrange("p (h d) -> p h d", h=HPG))

        def v_fin(rt, vps):
            if with_bias:
                nc.tensor.matmul(
                    vps, ones_sb[0:1, 0:P], bv_sb[0:1, :],
                    start=False, stop=True)
            nc.vector.tensor_copy(
                va_t[rt][:].rearrange("p (h d) -> p h d", h=HPG)[:, :, 0:D],
                vps.rearrange("p (h d) -> p h d", h=HPG))

        # ================= phase V1 + QK0: interleaved per-et =============
        # v1 covers rt 0..2 only: 3 + 2 + 2 PSUM tiles leaves one p_ps buf
        # free so v2(3) can start the moment the et-loop drains, covering
        # the vector copies that everything else gates on.
        NV1 = 3
        v_ps = [p_ps.tile([P, EG], F32, tag="ps", name=f"vps{rt}")
                for rt in range(NV1)]
        q0 = p_pr.tile([P, S], F32, tag="pr", name="q0pr")
        k0 = p_pr.tile([P, S], F32, tag="pr", name="k0pr")
        for et in range(ET):
            for rt in range(NV1):
                nc.tensor.matmul(
                    v_ps[rt][:], xt_t[et][:, rt * P:(rt + 1) * P], wv_t[et][:],
                    start=(et == 0), stop=(et == ET - 1 and not with_bias))
            for rc in range(NQC):  # rc-outer: xt half 1 consumed last
                nc.tensor.matmul(
                    q0[:, rc * QCH:(rc + 1) * QCH],
                    wq_t[et][:, 0:P], xt_t[et][:, rc * QCH:(rc + 1) * QCH],
                    start=(et == 0), stop=(et == ET - 1))
                nc.tensor.matmul(
                    k0[:, rc * QCH:(rc + 1) * QCH],
                    wk_t[et][:, 0:P], xt_t[et][:, rc * QCH:(rc + 1) * QCH],
                    start=(et == 0), stop=(et == ET - 1))

        def qk_copy(dst, src, bias_sb, ct):
            if with_bias:
                nc.vector.tensor_scalar_add(dst, src, bias_sb[:, ct:ct + 1])
            else:
                nc.vector.tensor_copy(dst, src)

        # scores gate on these two copies -> they go first on vector
        qk_copy(qT_t[0][:, 0:S], q0[:], bq_sb, 0)
        qk_copy(kT_t[0][:, 0:S], k0[:], bk_sb, 0)
        for rt in range(NV1):
            v_fin(rt, v_ps[rt][:])

        # ================= chunked units for the pipelined stream =========
        def v2_chunk(rt):
            vps = p_ps.tile([P, EG], F32, tag="ps", name=f"vps{rt}")
            for et in range(ET):
                nc.tensor.matmul(
                    vps[:], xt_t[et][:, rt * P:(rt + 1) * P], wv_t[et][:],
                    start=(et == 0), stop=(et == ET - 1 and not with_bias))
            v_fin(rt, vps[:])

        qk_state = {}

        def qk_part(ct, which, half):
            # which: 'q'|'k'; half 0 -> et 0..3, 1 -> et 4..7 (+copies)
            wt = wq_t if which == 'q' else wk_t
            if half == 0:
                qk_state[(ct, which)] = [
                    p_ps.tile([P, QCH], F32, tag="ps", name=f"{which}{ct}ps{rc}")
                    for rc in range(NQC)]
            accs = qk_state[(ct, which)]
            for et in range(half * 4, half * 4 + 4):
                for rc in range(NQC):
                    nc.tensor.matmul(
                        accs[rc][:], wt[et][:, ct * P:(ct + 1) * P],
                        xt_t[et][:, rc * QCH:(rc + 1) * QCH],
                        start=(et == 0), stop=(et == ET - 1))
            if half == 1:
                dstt = qT_t if which == 'q' else kT_t
                bias_sb = bq_sb if which == 'q' else bk_sb
                for rc in range(NQC):
                    qk_copy(dstt[ct][:, rc * QCH:(rc + 1) * QCH],
                            accs[rc][:], bias_sb, ct)

        # score units: ex_store[(qc, p, kt, h)] = (ex AP slice, off)
        ex_store = {}

        def s_unit(qc, p, kp):
            """Scores for head pair p, k-tiles (2kp, 2kp+1), q-window qc.
            The two heads' MMs interleave at row groups 0/64 and stream
            concurrently; per-head packed PSUM [0:na]+[na:na+nb]; one exp
            activate per head; diagonal masks on gpsimd."""
            kts = (2 * kp, 2 * kp + 1)
            offs = [max(0, kt - qc * (QCH // P)) * P for kt in kts]
            ns = [QCH - o for o in offs]
            cols = [0, ns[0]]
            w = ns[0] + ns[1]
            prs = [p_pr.tile([P, S], F32, tag="pr", name=f"pr{qc}{p}{kp}{h}")
                   for h in range(2)]
            exs = [p_exp.tile([P, S], BF16, tag="ex", name=f"ex{qc}{p}{kp}{h}")
                   for h in range(2)]
            for i, kt in enumerate(kts):
                for h in range(2):
                    hb = h * D
                    nc.tensor.matmul(
                        prs[h][:, cols[i]:cols[i] + ns[i]],
                        kT_t[p][hb:hb + D, kt * P:(kt + 1) * P],
                        qT_t[p][hb:hb + D,
                                qc * QCH + offs[i]:(qc + 1) * QCH],
                        start=True, stop=True,
                        tile_position=(hb, 0))
            for h in range(2):
                nc.scalar.activation(exs[h][:, 0:w], prs[h][:, 0:w], AF.Exp)
            for h in range(2):  # h-outer: h0 masks drain during h1's exp
                for i, kt in enumerate(kts):
                    if kt >= qc * (QCH // P):  # diagonal-crossing tile
                        nc.gpsimd.tensor_mul(
                            exs[h][:, cols[i]:cols[i] + P],
                            exs[h][:, cols[i]:cols[i] + P], mask_sb[:])
            for i, kt in enumerate(kts):
                for h in range(2):
                    ex_store[(qc, p, kt, h)] = (
                        exs[h][:, cols[i]:cols[i] + ns[i]], offs[i])

        def av_unit(qc, p, h):
            """Context accumulation for one head (global head 2p+h)."""
            gh = 2 * p + h
            hb = h * D
            av = p_ps.tile([P, QCH], F32, tag="ps", name=f"av{qc}{p}{h}")
            n_kt = (qc + 1) * (QCH // P)
            for kt in range(n_kt):
                ex_ap, off = ex_store[(qc, p, kt, h)]
                nc.tensor.matmul(
                    av[:, off:QCH],
                    va_t[kt][:, gh * 2 * D:(gh + 1) * 2 * D],
                    ex_ap,
                    start=(kt == 0), stop=(kt == n_kt - 1))
            den_sb = p_rc.tile([D, QCH], F32, tag="den", name="den")
            nc.vector.tensor_copy(den_sb[:], av[D:2 * D, :])
            rcb = p_rc.tile([D, QCH], F32, tag="rc", name="rc")
            nc.vector.reciprocal_approx_fast(rcb[:], den_sb[:])
            nc.vector.tensor_mul(
                ctx_t[p][hb:hb + D, qc * QCH:(qc + 1) * QCH],
                av[0:D, :], rcb[:])

        def project(rt):
            for cc in range(E // QCH):
                ps = p_ps.tile([P, QCH], F32, tag="ps", name=f"pj{rt}{cc}")
                for et in range(CT):
                    nc.tensor.matmul(
                        ps[:],
                        ctx_t[et][:, rt * P:(rt + 1) * P],
                        wp_t[et][:, cc * QCH:(cc + 1) * QCH],
                        start=(et == 0),
                        stop=(et == CT - 1 and not with_bias))
                if with_bias:
                    nc.tensor.matmul(
                        ps[:], ones_sb[0:1, 0:P],
                        bp_sb[0:1, cc * QCH:(cc + 1) * QCH],
                        start=False, stop=True)
                osb = p_osb.tile([P, QCH], BF16, tag="osb", name="osb")
                if (rt + cc) % 2 == 0:
                    nc.vector.tensor_copy(osb[:], ps[:])
                else:
                    nc.scalar.copy(osb[:], ps[:])
                # gpsimd is busy with masks mid-kernel but free in the tail
                eng = nc.gpsimd if rt >= 6 and (rt + cc) % 2 == 1 else nc.sync
                eng.dma_start(
                    out[rt * P:(rt + 1) * P, cc * QCH:(cc + 1) * QCH],
                    osb[:])

        # ================= the pipelined stream ===========================
        # block 0: att0/att1 scores of pair 0; fillers v2 + qk1.
        # v2(3) leads into the free PSUM buf while vector runs the copies.
        v2_chunk(3)
        s_unit(0, 0, 0)
        v2_chunk(4)
        s_unit(0, 0, 1)
        v2_chunk(5)
        s_unit(1, 0, 0)
        v2_chunk(6)
        s_unit(1, 0, 1)
        v2_chunk(7)
        qk_part(1, 'k', 0)
        s_unit(1, 0, 2)
        qk_part(1, 'k', 1)
        s_unit(1, 0, 3)
        qk_part(1, 'q', 0)
        qk_part(1, 'q', 1)
        av_unit(0, 0, 0)
        av_unit(0, 0, 1)

        # block 1: pair 1 scores; fillers qk2 + av of pair 0
        s_unit(0, 1, 0)
        qk_part(2, 'k', 0)
        s_unit(0, 1, 1)
        av_unit(1, 0, 0)
        s_unit(1, 1, 0)
        av_unit(1, 0, 1)
        s_unit(1, 1, 1)
        qk_part(2, 'k', 1)
        s_unit(1, 1, 2)
        qk_part(2, 'q', 0)
        s_unit(1, 1, 3)
        qk_part(2, 'q', 1)
        av_unit(0, 1, 0)
        av_unit(0, 1, 1)

        # block 2: pair 2 scores; fillers qk3 + av(1,1,h0) (h1 deferred
        # to block 3, which otherwise runs out of exp-independent work)
        s_unit(0, 2, 0)
        qk_part(3, 'k', 0)
        s_unit(0, 2, 1)
        av_unit(1, 1, 0)
        s_unit(1, 2, 0)
        qk_part(3, 'k', 1)
        s_unit(1, 2, 1)
        qk_part(3, 'q', 0)
        s_unit(1, 2, 2)
        qk_part(3, 'q', 1)
        s_unit(1, 2, 3)
        av_unit(0, 2, 0)
        s_unit(1, 3, 0)   # pulled forward: exps drain in block2's slack
        av_unit(0, 2, 1)
        s_unit(1, 3, 1)

        # block 3: last pair; fillers av(1,1,h1) + av(1,2) + proj(0..3)
        s_unit(0, 3, 0)
        av_unit(1, 1, 1)
        s_unit(0, 3, 1)
        av_unit(1, 2, 0)
        av_unit(1, 2, 1)
        av_unit(0, 3, 0)
        av_unit(0, 3, 1)
        project(0)
        s_unit(1, 3, 2)
        project(1)
        s_unit(1, 3, 3)
        project(2)
        project(3)

        # tail
        av_unit(1, 3, 0)
        av_unit(1, 3, 1)
        for rt in range(4, RT):
            project(rt)


def build_nc(with_bias=False):
    nc = bacc.Bacc("TRN2", target_bir_lowering=False, debug=False)
    with tile.TileContext(nc) as tc, nc.allow_low_precision(
        reason="bf16 matmul operands with fp32 accumulate; approx reciprocal"
    ):
        _emit(nc, tc, with_bias)
    nc.compile()
    return nc


def make_in_maps(x, Wqkv, bqkv, Wproj, bproj):
    x = np.asarray(x, dtype=np.float32)
    Wqkv = np.asarray(Wqkv, dtype=np.float32)
    bqkv = np.asarray(bqkv, dtype=np.float32)
    Wproj = np.asarray(Wproj, dtype=np.float32)
    bproj = np.asarray(bproj, dtype=np.float32)
    mask = np.triu(np.ones((P, P), dtype=np.float32))  # [k, q]: k <= q
    in_maps = []
    for c in range(8):
        b, hg = c // 2, c % 2
        g = slice(hg * EG, (hg + 1) * EG)
        in_maps.append({
            "xt": np.ascontiguousarray(x[b].T).reshape(ET, P, S).astype(BF_NP),
            "wq": np.ascontiguousarray(
                Wqkv[:, 0 * E:1 * E][:, g] * SCALE).reshape(ET, P, EG).astype(BF_NP),
            "wk": np.ascontiguousarray(
                Wqkv[:, 1 * E:2 * E][:, g]).reshape(ET, P, EG).astype(BF_NP),
            "wv": np.ascontiguousarray(
                Wqkv[:, 2 * E:3 * E][:, g]).reshape(ET, P, EG).astype(BF_NP),
            "wp": np.ascontiguousarray(Wproj[g, :]).reshape(CT, P, E).astype(BF_NP),
            "bq": np.ascontiguousarray(
                (bqkv[0 * E:1 * E][g] * SCALE).reshape(CT, P).T),
            "bk": np.ascontiguousarray(
                bqkv[1 * E:2 * E][g].reshape(CT, P).T),
            "bv": bqkv[2 * E:3 * E][g].reshape(1, EG).astype(BF_NP),
            "bp": (bproj if hg == 0 else np.zeros_like(bproj)
                   ).reshape(1, E).astype(BF_NP),
            "mask": mask.astype(BF_NP),
            "ones": np.ones((1, P), dtype=BF_NP),
            "vones": np.ones((P, HPG * D), dtype=BF_NP),
        })
    return in_maps


def kernel(x, Wqkv, bqkv, Wproj, bproj):
    with_bias = bool(
        np.any(np.asarray(bqkv)) or np.any(np.asarray(bproj)))
    nc = build_nc(with_bias)
    in_maps = make_in_maps(x, Wqkv, bqkv, Wproj, bproj)
    res = run_bass_kernel_spmd(nc, in_maps, list(range(8))).results
    out = np.zeros((B, S, E), dtype=np.float32)
    for c in range(8):
        out[c // 2] += res[c]["out"]
    return out


# revision 32
# speedup vs baseline: 1.0152x; 1.0118x over previous
"""Causal multi-head attention on 8 trn2 NeuronCores.

Sharding: core c -> (batch b = c//2, head-group hg = c%2).
Each head-group owns 8 of the 16 heads (512 of the 1024 embed dims after
the head split). Per core:
  - qT, kT   = (x[b] @ Wq_hg)^T (wq pre-scaled by 1/sqrt(d)), (x[b] @ Wk_hg)^T
  - v        = x[b] @ Wv_hg, packed per-head with a ones column -> va
  - scoresT  = kT.T-contract [krows, qrows] tiles; exp (-> bf16); causal mask
  - av       = va^T @ ex -> [128, qcols]: rows 0:64 context, rows 64:128 the
               softmax denominator replicated 64x (va carries 64 ones cols, so
               the broadcast falls out of the matmul for free)
  - ctxT     = av[0:64] * approx_recip(av[64:128])
  - partial  = ctxT.T @ Wproj_hg (+ bproj on hg==0 cores only)
Host: out[b] = partial(b,0) + partial(b,1).

v2 schedule (vs baseline, 140.7us -> ~119us): the whole kernel is one
software-pipelined stream keyed to two facts from the baseline trace:
(1) the PE streams at its pure warm rate whenever its queue is
non-empty, and (2) scalar exp (~50us) is the only consumer that can
starve it. So:
  - v(rt0-2) + qk(ct0) interleave per-et so compute tracks the DMA
    stream and the first exp starts ~8us earlier; one PSUM buf is kept
    free so v2(3) bridges the post-loop copy latency.
  - input DMA: xt in halves, only ct0 columns of wq/wk early (3.5MB
    early stream), the remaining 4MB strictly behind it in-queue so it
    cannot steal bandwidth from the et-loop's feed.
  - score MMs for the two heads of a ct-tile issue interleaved at row
    groups (0,0)/(64,0) -> they stream concurrently (K=64 packing).
  - emission interleaves {score units, av chains, qk/v2/proj chunks} so
    there is >= the exp drain time of PE work between a score unit and
    anything that waits on its exp; PE never idles -> HAM stays warm
    for the whole kernel (one cold ramp instead of four).
  - causal masks moved scalar/vector -> gpsimd (otherwise idle).
  - all PSUM->SBUF copies moved off scalar (exp-only) onto vector.
"""

import sys

try:
    import concourse.bass as bass  # noqa: F401
except Exception:
    sys.path.insert(0, "/opt/trn_rl_repo")

import numpy as np
import ml_dtypes

import concourse.bass as bass
import concourse.mybir as mybir
import concourse.tile as tile
from concourse import bacc
from concourse.bass_utils import run_bass_kernel_spmd

F32 = mybir.dt.float32
F32R = mybir.dt.float32r
BF16 = mybir.dt.bfloat16
AF = mybir.ActivationFunctionType
BF_NP = ml_dtypes.bfloat16

B, S, E = 4, 1024, 1024
H, D = 16, 64
HG = 2              # head groups (cores per batch)
HPG = H // HG       # 8 heads per group
EG = HPG * D        # 512 embed cols per group
P = 128
ET = E // P         # 8 embed tiles
RT = S // P         # 8 row tiles
CT = EG // P        # 4 col tiles of the group's q/k
QCH = 512           # q-chunk (moving free dim)
NQC = S // QCH      # 2 q chunks
SCALE = 1.0 / np.sqrt(D)


def _emit(nc, tc, with_bias):
    xt = nc.dram_tensor("xt", [ET, P, S], BF16, kind="ExternalInput")
    wq = nc.dram_tensor("wq", [ET, P, EG], BF16, kind="ExternalInput")
    wk = nc.dram_tensor("wk", [ET, P, EG], BF16, kind="ExternalInput")
    wv = nc.dram_tensor("wv", [ET, P, EG], BF16, kind="ExternalInput")
    wp = nc.dram_tensor("wp", [CT, P, E], BF16, kind="ExternalInput")
    bq = nc.dram_tensor("bq", [P, CT], F32, kind="ExternalInput")
    bk = nc.dram_tensor("bk", [P, CT], F32, kind="ExternalInput")
    bv = nc.dram_tensor("bv", [1, EG], BF16, kind="ExternalInput")
    bp = nc.dram_tensor("bp", [1, E], BF16, kind="ExternalInput")
    mask = nc.dram_tensor("mask", [P, P], BF16, kind="ExternalInput")
    ones_in = nc.dram_tensor("ones", [1, P], BF16, kind="ExternalInput")
    vones_in = nc.dram_tensor("vones", [P, HPG * D], BF16, kind="ExternalInput")
    out = nc.dram_tensor("out", [S, E], BF16, kind="ExternalOutput")

    with (
        tc.tile_pool(name="xt", bufs=1) as p_xt,
        tc.tile_pool(name="wqkv", bufs=1) as p_w,
        tc.tile_pool(name="wp", bufs=1) as p_wp,
        tc.tile_pool(name="qt", bufs=1) as p_qt,
        tc.tile_pool(name="kt", bufs=1) as p_kt,
        tc.tile_pool(name="vaug", bufs=1) as p_va,
        tc.tile_pool(name="ctxT", bufs=1) as p_ctx,
        tc.tile_pool(name="exps", bufs=24) as p_exp,
        tc.tile_pool(name="small", bufs=1) as p_sm,
        tc.tile_pool(name="recip", bufs=6) as p_rc,
        tc.tile_pool(name="osb", bufs=6) as p_osb,
        tc.tile_pool(name="pr", bufs=2, space="PSUM") as p_pr,   # 2x[128,1024]f32
        tc.tile_pool(name="ps1", bufs=4, space="PSUM") as p_ps,  # 4x[128,512]f32
    ):
        # ---- SBUF input tiles ----
        xt_t = [p_xt.tile([P, S], BF16, tag=f"xt{et}", name=f"xt{et}")
                for et in range(ET)]
        wq_t = [p_w.tile([P, EG], BF16, tag=f"wq{et}", name=f"wq{et}")
                for et in range(ET)]
        wk_t = [p_w.tile([P, EG], BF16, tag=f"wk{et}", name=f"wk{et}")
                for et in range(ET)]
        wv_t = [p_w.tile([P, EG], BF16, tag=f"wv{et}", name=f"wv{et}")
                for et in range(ET)]
        wp_t = [p_wp.tile([P, E], BF16, tag=f"wp{et}", name=f"wp{et}")
                for et in range(CT)]

        # ---- input DMA: per-et groups, xt in halves, so the v1/qk0 et-loop
        # tracks the stream; spread across the three DMA-capable queues.
        # Only ct=0 columns of wq/wk ship early (all qk0 needs): the early
        # stream is 3.5MB instead of 6MB, so compute tracks DMA ~5us sooner.
        # Queue plan (transfers within a queue are ordered, queues share HBM
        # bandwidth ~equally): sync: xt halves 0 (1MB); scalar: wv (1MB);
        # gpsimd: wq/wk ct0 + xt halves 1 (1.5MB). The late 4MB (wq/wk ct1-3,
        # wp) sits strictly BEHIND the early stream in its queue so it cannot
        # steal bandwidth from the et-loop's feed.
        nc.sync.dma_start(xt_t[0][:, 0:2 * P], xt[0][:, 0:2 * P])
        nc.sync.dma_start(xt_t[0][:, 2 * P:QCH], xt[0][:, 2 * P:QCH])
        for et in range(ET):
            nc.scalar.dma_start(wv_t[et][:], wv[et])
            if et > 0:
                nc.sync.dma_start(xt_t[et][:, 0:QCH], xt[et][:, 0:QCH])
            nc.gpsimd.dma_start(wk_t[et][:, 0:P], wk[et][:, 0:P])
            nc.gpsimd.dma_start(wq_t[et][:, 0:P], wq[et][:, 0:P])
            nc.gpsimd.dma_start(xt_t[et][:, QCH:S], xt[et][:, QCH:S])
        mask_sb = p_sm.tile([P, P], BF16, tag="mask", name="maskt")
        nc.gpsimd.dma_start(mask_sb[:], mask[:])
        vones_sb = p_sm.tile([P, HPG * D], BF16, tag="vones", name="vones")
        nc.gpsimd.dma_start(vones_sb[:], vones_in[:])
        # late stream: remaining wq/wk columns (ct 1..3) + wp
        for et in range(ET):
            nc.scalar.dma_start(wq_t[et][:, P:EG], wq[et][:, P:EG])
            nc.gpsimd.dma_start(wk_t[et][:, P:EG], wk[et][:, P:EG])
        for et in range(CT):
            nc.sync.dma_start(wp_t[et][:], wp[et])
        if with_bias:
            ones_sb = p_sm.tile([1, P], BF16, tag="ones", name="ones")
            nc.sync.dma_start(ones_sb[:], ones_in[:])
            bq_sb = p_sm.tile([P, CT], F32, tag="bq", name="bqt")
            nc.sync.dma_start(bq_sb[:], bq[:])
            bk_sb = p_sm.tile([P, CT], F32, tag="bk", name="bkt")
            nc.sync.dma_start(bk_sb[:], bk[:])
            bv_sb = p_sm.tile([1, EG], BF16, tag="bv", name="bvt")
            nc.sync.dma_start(bv_sb[:], bv[:])
            bp_sb = p_sm.tile([1, E], BF16, tag="bp", name="bpt")
            nc.sync.dma_start(bp_sb[:], bp[:])
        else:
            ones_sb = bq_sb = bk_sb = bv_sb = bp_sb = None

        # ---- long-lived SBUF result tiles ----
        qT_t = [p_qt.tile([P, S], BF16, tag=f"qt{ct}", name=f"qt{ct}")
                for ct in range(CT)]
        kT_t = [p_kt.tile([P, S], BF16, tag=f"kt{ct}", name=f"kt{ct}")
                for ct in range(CT)]
        va_t = [p_va.tile([P, HPG * 2 * D], BF16, tag=f"va{rt}", name=f"va{rt}")
                for rt in range(RT)]
        ctx_t = [p_ctx.tile([P, S], BF16, tag=f"cx{i}", name=f"cx{i}")
                 for i in range(CT)]

        # ones-halves of va (vector, early while otherwise idle)
        for rt in range(RT):
            va3 = va_t[rt][:].rearrange("p (h d) -> p h d", h=HPG)
            nc.vector.tensor_copy(
                va3[:, :, D:2 * D],
                vones_sb[:].rearrange("p (h d) -> p h d", h=HPG))

        def v_fin(rt, vps):
            if with_bias:
                nc.tensor.matmul(
                    vps, ones_sb[0:1, 0:P], bv_sb[0:1, :],
                    start=False, stop=True)
            nc.vector.tensor_copy(
                va_t[rt][:].rearrange("p (h d) -> p h d", h=HPG)[:, :, 0:D],
                vps.rearrange("p (h d) -> p h d", h=HPG))

        # ================= phase V1 + QK0: interleaved per-et =============
        # v1 covers rt 0..2 only: 3 + 2 + 2 PSUM tiles leaves one p_ps buf
        # free so v2(3) can start the moment the et-loop drains, covering
        # the vector copies that everything else gates on.
        NV1 = 3
        v_ps = [p_ps.tile([P, EG], F32, tag="ps", name=f"vps{rt}")
                for rt in range(NV1)]
        q0 = p_pr.tile([P, S], F32, tag="pr", name="q0pr")
        k0 = p_pr.tile([P, S], F32, tag="pr", name="k0pr")
        for et in range(ET):
            for rt in range(NV1):
                nc.tensor.matmul(
                    v_ps[rt][:], xt_t[et][:, rt * P:(rt + 1) * P], wv_t[et][:],
                    start=(et == 0), stop=(et == ET - 1 and not with_bias))
            for rc in range(NQC):  # rc-outer: xt half 1 consumed last
                nc.tensor.matmul(
                    q0[:, rc * QCH:(rc + 1) * QCH],
                    wq_t[et][:, 0:P], xt_t[et][:, rc * QCH:(rc + 1) * QCH],
                    start=(et == 0), stop=(et == ET - 1))
                nc.tensor.matmul(
                    k0[:, rc * QCH:(rc + 1) * QCH],
                    wk_t[et][:, 0:P], xt_t[et][:, rc * QCH:(rc + 1) * QCH],
                    start=(et == 0), stop=(et == ET - 1))

        def qk_copy(dst, src, bias_sb, ct):
            if with_bias:
                nc.vector.tensor_scalar_add(dst, src, bias_sb[:, ct:ct + 1])
            else:
                nc.vector.tensor_copy(dst, src)

        # scores gate on these two copies -> they go first on vector
        qk_copy(qT_t[0][:, 0:S], q0[:], bq_sb, 0)
        qk_copy(kT_t[0][:, 0:S], k0[:], bk_sb, 0)
        for rt in range(NV1):
            v_fin(rt, v_ps[rt][:])

        # ================= chunked units for the pipelined stream =========
        def v2_chunk(rt):
            vps = p_ps.tile([P, EG], F32, tag="ps", name=f"vps{rt}")
            for et in range(ET):
                nc.tensor.matmul(
                    vps[:], xt_t[et][:, rt * P:(rt + 1) * P], wv_t[et][:],
                    start=(et == 0), stop=(et == ET - 1 and not with_bias))
            v_fin(rt, vps[:])

        qk_state = {}

        def qk_part(ct, which, half):
            # which: 'q'|'k'; half 0 -> et 0..3, 1 -> et 4..7 (+copies)
            wt = wq_t if which == 'q' else wk_t
            if half == 0:
                qk_state[(ct, which)] = [
                    p_ps.tile([P, QCH], F32, tag="ps", name=f"{which}{ct}ps{rc}")
                    for rc in range(NQC)]
            accs = qk_state[(ct, which)]
            for et in range(half * 4, half * 4 + 4):
                for rc in range(NQC):
                    nc.tensor.matmul(
                        accs[rc][:], wt[et][:, ct * P:(ct + 1) * P],
                        xt_t[et][:, rc * QCH:(rc + 1) * QCH],
                        start=(et == 0), stop=(et == ET - 1))
            if half == 1:
                dstt = qT_t if which == 'q' else kT_t
                bias_sb = bq_sb if which == 'q' else bk_sb
                for rc in range(NQC):
                    qk_copy(dstt[ct][:, rc * QCH:(rc + 1) * QCH],
                            accs[rc][:], bias_sb, ct)

        # score units: ex_store[(qc, p, kt, h)] = (ex AP slice, off)
        ex_store = {}

        def s_unit(qc, p, kp):
            """Scores for head pair p, k-tiles (2kp, 2kp+1), q-window qc.
            The two heads' MMs interleave at row groups 0/64 and stream
            concurrently; per-head packed PSUM [0:na]+[na:na+nb]; one exp
            activate per head; diagonal masks on gpsimd."""
            kts = (2 * kp, 2 * kp + 1)
            offs = [max(0, kt - qc * (QCH // P)) * P for kt in kts]
            ns = [QCH - o for o in offs]
            cols = [0, ns[0]]
            w = ns[0] + ns[1]
            prs = [p_pr.tile([P, S], F32, tag="pr", name=f"pr{qc}{p}{kp}{h}")
                   for h in range(2)]
            exs = [p_exp.tile([P, S], BF16, tag="ex", name=f"ex{qc}{p}{kp}{h}")
                   for h in range(2)]
            for i, kt in enumerate(kts):
                for h in range(2):
                    hb = h * D
                    nc.tensor.matmul(
                        prs[h][:, cols[i]:cols[i] + ns[i]],
                        kT_t[p][hb:hb + D, kt * P:(kt + 1) * P],
                        qT_t[p][hb:hb + D,
                                qc * QCH + offs[i]:(qc + 1) * QCH],
                        start=True, stop=True,
                        tile_position=(hb, 0))
            for h in range(2):
                nc.scalar.activation(exs[h][:, 0:w], prs[h][:, 0:w], AF.Exp)
            for h in range(2):  # h-outer: h0 masks drain during h1's exp
                for i, kt in enumerate(kts):
                    if kt >= qc * (QCH // P):  # diagonal-crossing tile
                        nc.gpsimd.tensor_mul(
                            exs[h][:, cols[i]:cols[i] + P],
                            exs[h][:, cols[i]:cols[i] + P], mask_sb[:])
            for i, kt in enumerate(kts):
                for h in range(2):
                    ex_store[(qc, p, kt, h)] = (
                        exs[h][:, cols[i]:cols[i] + ns[i]], offs[i])

        def av_unit(qc, p, h):
            """Context accumulation for one head (global head 2p+h)."""
            gh = 2 * p + h
            hb = h * D
            av = p_ps.tile([P, QCH], F32, tag="ps", name=f"av{qc}{p}{h}")
            n_kt = (qc + 1) * (QCH // P)
            for kt in range(n_kt):
                ex_ap, off = ex_store[(qc, p, kt, h)]
                nc.tensor.matmul(
                    av[:, off:QCH],
                    va_t[kt][:, gh * 2 * D:(gh + 1) * 2 * D],
                    ex_ap,
                    start=(kt == 0), stop=(kt == n_kt - 1))
            den_sb = p_rc.tile([D, QCH], F32, tag="den", name="den")
            nc.vector.tensor_copy(den_sb[:], av[D:2 * D, :])
            rcb = p_rc.tile([D, QCH], F32, tag="rc", name="rc")
            nc.vector.reciprocal_approx_fast(rcb[:], den_sb[:])
            nc.vector.tensor_mul(
                ctx_t[p][hb:hb + D, qc * QCH:(qc + 1) * QCH],
                av[0:D, :], rcb[:])

        def project(rt):
            for cc in range(E // QCH):
                ps = p_ps.tile([P, QCH], F32, tag="ps", name=f"pj{rt}{cc}")
                for et in range(CT):
                    nc.tensor.matmul(
                        ps[:],
                        ctx_t[et][:, rt * P:(rt + 1) * P],
                        wp_t[et][:, cc * QCH:(cc + 1) * QCH],
                        start=(et == 0),
                        stop=(et == CT - 1 and not with_bias))
                if with_bias:
                    nc.tensor.matmul(
                        ps[:], ones_sb[0:1, 0:P],
                        bp_sb[0:1, cc * QCH:(cc + 1) * QCH],
                        start=False, stop=True)
                osb = p_osb.tile([P, QCH], BF16, tag="osb", name="osb")
                if (rt + cc) % 2 == 0:
                    nc.vector.tensor_copy(osb[:], ps[:])
                else:
                    nc.scalar.copy(osb[:], ps[:])
                # gpsimd is busy with masks mid-kernel but free in the tail
                eng = nc.gpsimd if rt >= 6 and (rt + cc) % 2 == 1 else nc.sync
                eng.dma_start(
                    out[rt * P:(rt + 1) * P, cc * QCH:(cc + 1) * QCH],
                    osb[:])

        # ================= the pipelined stream ===========================
        # block 0: att0/att1 scores of pair 0; fillers v2 + qk1.
        # v2(3) leads into the free PSUM buf while vector runs the copies.
        v2_chunk(3)
        s_unit(0, 0, 0)
        v2_chunk(4)
        s_unit(0, 0, 1)
        v2_chunk(5)
        s_unit(1, 0, 0)
        v2_chunk(6)
        s_unit(1, 0, 1)
        v2_chunk(7)
        qk_part(1, 'k', 0)
        s_unit(1, 0, 2)
        qk_part(1, 'k', 1)
        s_unit(1, 0, 3)
        qk_part(1, 'q', 0)
        qk_part(1, 'q', 1)
        av_unit(0, 0, 0)
        av_unit(0, 0, 1)

        # block 1: pair 1 scores; fillers qk2 + av of pair 0
        s_unit(0, 1, 0)
        qk_part(2, 'k', 0)
        s_unit(0, 1, 1)
        av_unit(1, 0, 0)
        s_unit(1, 1, 0)
        av_unit(1, 0, 1)
        s_unit(1, 1, 1)
        qk_part(2, 'k', 1)
        s_unit(1, 1, 2)
        qk_part(2, 'q', 0)
        s_unit(1, 1, 3)
        qk_part(2, 'q', 1)
        av_unit(0, 1, 0)
        av_unit(0, 1, 1)

        # block 2: pair 2 scores; fillers qk3 + av(1,1,h0) (h1 deferred
        # to block 3, which otherwise runs out of exp-independent work)
        s_unit(0, 2, 0)
        qk_part(3, 'k', 0)
        s_unit(0, 2, 1)
        av_unit(1, 1, 0)
        s_unit(1, 2, 0)
        qk_part(3, 'k', 1)
        s_unit(1, 2, 1)
        qk_part(3, 'q', 0)
        s_unit(1, 2, 2)
        qk_part(3, 'q', 1)
        s_unit(1, 2, 3)
        av_unit(0, 2, 0)
        s_unit(1, 3, 0)   # pulled forward: exps drain in block2's slack
        av_unit(0, 2, 1)
        s_unit(1, 3, 1)

        # block 3: last pair; fillers av(1,1,h1) + av(1,2) + proj(0..3)
        s_unit(0, 3, 0)
        av_unit(1, 1, 1)
        s_unit(0, 3, 1)
        av_unit(1, 2, 0)
        av_unit(1, 2, 1)
        av_unit(0, 3, 0)
        av_unit(0, 3, 1)
        project(0)
        s_unit(1, 3, 2)
        project(1)
        s_unit(1, 3, 3)
        project(2)
        project(3)

        # tail
        av_unit(1, 3, 0)
        av_unit(1, 3, 1)
        for rt in range(4, RT):
            project(rt)


def build_nc(with_bias=False):
    nc = bacc.Bacc("TRN2", target_bir_lowering=False, debug=False)
    with tile.TileContext(nc) as tc, nc.allow_low_precision(
        reason="bf16 matmul operands with fp32 accumulate; approx reciprocal"
    ):
        _emit(nc, tc, with_bias)
    nc.compile()
    return nc


def make_in_maps(x, Wqkv, bqkv, Wproj, bproj):
    x = np.asarray(x, dtype=np.float32)
    Wqkv = np.asarray(Wqkv, dtype=np.float32)
    bqkv = np.asarray(bqkv, dtype=np.float32)
    Wproj = np.asarray(Wproj, dtype=np.float32)
    bproj = np.asarray(bproj, dtype=np.float32)
    mask = np.triu(np.ones((P, P), dtype=np.float32))  # [k, q]: k <= q
    in_maps = []
    for c in range(8):
        b, hg = c // 2, c % 2
        g = slice(hg * EG, (hg + 1) * EG)
        in_maps.append({
            "xt": np.ascontiguousarray(x[b].T).reshape(ET, P, S).astype(BF_NP),
            "wq": np.ascontiguousarray(
                Wqkv[:, 0 * E:1 * E][:, g] * SCALE).reshape(ET, P, EG).astype(BF_NP),
            "wk": np.ascontiguousarray(
                Wqkv[:, 1 * E:2 * E][:, g]).reshape(ET, P, EG).astype(BF_NP),
            "wv": np.ascontiguousarray(
                Wqkv[:, 2 * E:3 * E][:, g]).reshape(ET, P, EG).astype(BF_NP),
            "wp": np.ascontiguousarray(Wproj[g, :]).reshape(CT, P, E).astype(BF_NP),
            "bq": np.ascontiguousarray(
                (bqkv[0 * E:1 * E][g] * SCALE).reshape(CT, P).T),
            "bk": np.ascontiguousarray(
                bqkv[1 * E:2 * E][g].reshape(CT, P).T),
            "bv": bqkv[2 * E:3 * E][g].reshape(1, EG).astype(BF_NP),
            "bp": (bproj if hg == 0 else np.zeros_like(bproj)
                   ).reshape(1, E).astype(BF_NP),
            "mask": mask.astype(BF_NP),
            "ones": np.ones((1, P), dtype=BF_NP),
            "vones": np.ones((P, HPG * D), dtype=BF_NP),
        })
    return in_maps


def kernel(x, Wqkv, bqkv, Wproj, bproj):
    with_bias = bool(
        np.any(np.asarray(bqkv)) or np.any(np.asarray(bproj)))
    nc = build_nc(with_bias)
    in_maps = make_in_maps(x, Wqkv, bqkv, Wproj, bproj)
    res = run_bass_kernel_spmd(nc, in_maps, list(range(8))).results
    out = np.zeros((B, S, E), dtype=np.float32)
    for c in range(8):
        out[c // 2] += res[c]["out"]
    return out
